# revision 5
# baseline (speedup 1.0000x reference)
"""BitLinear (ternary-quantized linear) Trainium2 kernel.

Computes: out = x @ dequant(weight).T where dequant is per-group(128)
AbsMean ternary quantization (w_q in {-1,0,+1} times per-group scale).

Strategy (8 NeuronCores, column-parallel / tensor-parallel):
  - weight [O=11008, K=4096] sharded by rows across 8 cores (1376 each).
  - x [T=8192, K] replicated, host-packed to the transposed tile layout
    AND host-cast to fp16, so each t-tile loads as one contiguous 1MB
    DMA straight into the matmul operand layout (no on-chip cast).
  - On-chip dequant per o-tile (128 rows), engine-balanced:
      DVE:    per-group abs-sum reduce, thresholds, c+ = (w > s/2),
              d = c+ - c-  (ternary in fp16)
      GpSimd: c- = (w < -s/2), w_eff = d * fp16(s) broadcast mult
      PE:     [o,k]->[k,o] transposes (identity matmul)
      ACT:    transpose evacuations into the resident ko-major tiles
  - Resident weight: 5 ko-major fp16 tiles of 256/256/256/256/352
    columns (rhs streams of 256+ cols hide LDWEIGHTS).
  - Warmup: as each weight tile finishes dequant, it is swept over the
    first WARM_T t-tiles (psum-accumulated per (t, wtile), partial-col
    DMA out).  No catch-up visits: by the time steady visits start all
    weight tiles are resident, so t-tiles 4..63 are single full visits.
  - Steady visit: per psum pool, pool-contiguous matmul runs
    (ko-major within the pool) to minimize pool switching; PSUM: two
    256-wide tiles accumulate into one 2KB bank; the second uses
    start=False at ko==0 and lands on the pending-zeroed bank.
  - Per-core output [T, 1376]; host concatenates along O.
"""

import os

import numpy as np

import concourse.bass as bass
import concourse.mybir as mybir
import concourse.tile as tile
from concourse import bacc
from concourse.bass_utils import run_bass_kernel_spmd
from concourse.masks import make_identity

P = 128
GROUP = 128
EPS = 1e-8
TB = 8

FULL_B, FULL_S, FULL_K, FULL_O = 4, 2048, 4096, 11008
N_CORES = 8

LAST_RESULT = None  # BassKernelResults of the most recent run (for test.py)

# Weight-tile column widths (>=256 so rhs streams hide LDWEIGHTS).
WTILE_COLS = [256, 256, 256, 256, 352]
# psum pool index for each weight tile (two 256s share one bank).
WTILE_POOL = [0, 0, 1, 1, 2]
# number of t-tiles each weight tile is swept over during warmup
WARM_T = 4


def build_program(K, T, O_SHARD, mm_dt=mybir.dt.float16):
    assert K % GROUP == 0 and T % P == 0
    KO = K // GROUP
    KH = KO // 2
    n_ttiles = T // P
    o_tiles = [(o0, min(P, O_SHARD - o0)) for o0 in range(0, O_SHARD, P)]
    n_ot = len(o_tiles)
    assert sum(WTILE_COLS) == O_SHARD
    wt_off = [sum(WTILE_COLS[:i]) for i in range(len(WTILE_COLS))]
    # o-tiles belonging to each weight tile
    wt_otiles = [
        [i for i, (o0, _) in enumerate(o_tiles)
         if c0 <= o0 < c0 + csz]
        for c0, csz in zip(wt_off, WTILE_COLS)
    ]

    nc = bacc.Bacc("TRN2", target_bir_lowering=False, debug=False)
    xt = nc.dram_tensor("xt", [T, K], mm_dt, kind="ExternalInput").ap()
    w = nc.dram_tensor(
        "w", [O_SHARD, K], mybir.dt.float32, kind="ExternalInput"
    ).ap()
    out = nc.dram_tensor(
        "out", [T, O_SHARD], mybir.dt.float32, kind="ExternalOutput"
    ).ap()

    with tile.TileContext(nc) as tc:
        with (
            tc.tile_pool(name="wres", bufs=1) as wres,
            tc.tile_pool(name="const", bufs=1) as constp,
            tc.tile_pool(name="deq32", bufs=2) as deq32,
            tc.tile_pool(name="deq16", bufs=4) as deq16,
            tc.tile_pool(name="tiny", bufs=3) as tiny,
            tc.tile_pool(name="xwarm", bufs=WARM_T) as xwarm,
            tc.tile_pool(name="xin", bufs=2) as xin,
            tc.tile_pool(name="outp", bufs=2) as outp,
            tc.tile_pool(name="stg", bufs=2) as stgp,
            tc.tile_pool(name="ps_tp", bufs=2, space="PSUM") as ps_tp,
            tc.tile_pool(name="ps_a", bufs=2, space="PSUM") as ps_a,
            tc.tile_pool(name="ps_b", bufs=2, space="PSUM") as ps_b,
            tc.tile_pool(name="ps_c", bufs=2, space="PSUM") as ps_c,
        ):
            ps_pools = [ps_a, ps_b, ps_c]
            pool_w = [512, 512, 352]
            pool_lo = [0, 512, 1024]
            # Resident dequantized transposed weight, ko-major per wtile:
            # wbt[wi][p, ko, col] with contiguous columns per ko (fast rhs).
            wbt = [
                wres.tile([P, KO, csz], mm_dt, tag=f"wbt{wi}", name=f"wbt{wi}")
                for wi, csz in enumerate(WTILE_COLS)
            ]
            ident = constp.tile([P, P], mm_dt)
            make_identity(nc, ident)

            # ------------- dequant of one o-tile (in ko-halves) -----------
            def emit_deq(i):
                o0, osz = o_tiles[i]
                wi = next(
                    j for j, c0 in enumerate(wt_off)
                    if c0 <= o0 < c0 + WTILE_COLS[j]
                )
                lo = o0 - wt_off[wi]
                wsrc = w[o0 : o0 + osz].rearrange("o (ko k) -> o ko k", k=GROUP)
                for h in range(2):
                    ka = h * KH
                    wt = deq32.tile([P, KH, GROUP], mybir.dt.float32,
                                    tag="wt", name="wt")
                    cp = deq16.tile([P, KH, GROUP], mm_dt, tag="cp", name="cp")
                    cm = deq16.tile([P, KH, GROUP], mm_dt, tag="cm", name="cm")
                    sums = tiny.tile([P, KH], mybir.dt.float32, tag="sums")
                    tpos = tiny.tile([P, KH], mybir.dt.float32, tag="tpos")
                    tneg = tiny.tile([P, KH], mybir.dt.float32, tag="tneg")
                    s16 = tiny.tile([P, KH], mm_dt, tag="s16")
                    nc.sync.dma_start(wt[:osz], wsrc[:, ka : ka + KH])
                    nc.vector.tensor_reduce(
                        sums[:osz], wt[:osz],
                        axis=mybir.AxisListType.X, op=mybir.AluOpType.add,
                        apply_absolute_value=True,
                    )
                    nc.vector.tensor_scalar(
                        tpos[:osz], sums[:osz], 0.5 / GROUP, 0.5 * EPS,
                        mybir.AluOpType.mult, mybir.AluOpType.max,
                    )
                    nc.vector.tensor_scalar(
                        tneg[:osz], sums[:osz], -0.5 / GROUP, -0.5 * EPS,
                        mybir.AluOpType.mult, mybir.AluOpType.min,
                    )
                    nc.vector.tensor_scalar(
                        s16[:osz], sums[:osz], 1.0 / GROUP, EPS,
                        mybir.AluOpType.mult, mybir.AluOpType.max,
                    )
                    nc.vector.tensor_tensor(
                        cp[:osz], wt[:osz],
                        tpos[:osz, :, None].to_broadcast((osz, KH, GROUP)),
                        mybir.AluOpType.is_gt,
                    )
                    nc.vector.tensor_tensor(
                        cm[:osz], wt[:osz],
                        tneg[:osz, :, None].to_broadcast((osz, KH, GROUP)),
                        mybir.AluOpType.is_lt,
                    )
                    nc.vector.tensor_tensor(
                        cp[:osz], cp[:osz], cm[:osz],
                        mybir.AluOpType.subtract,
                    )
                    nc.gpsimd.tensor_tensor(
                        cp[:osz], cp[:osz],
                        s16[:osz, :, None].to_broadcast((osz, KH, GROUP)),
                        mybir.AluOpType.mult,
                    )
                    for g in range(KH // TB):
                        kb = ka + g * TB
                        ps = ps_tp.tile([P, TB, P], mm_dt, tag="tp")
                        for j in range(TB):
                            nc.tensor.transpose(
                                ps[:, j, :osz], cp[:osz, g * TB + j, :],
                                ident[:osz, :osz],
                            )
                        nc.scalar.copy(
                            wbt[wi][:, kb : kb + TB, lo : lo + osz],
                            ps[:, :, :osz],
                        )

            # ------------- x tile load ------------------------------------
            xt_r = xt.rearrange("(tt p) (ko t) -> tt p ko t", p=P, t=P)

            def load_x(tt, pool):
                xb = pool.tile([P, KO, P], mm_dt, tag="xb")
                nc.sync.dma_start(xb, xt_r[tt])
                return xb

            # ------------- warmup sweep: one (t-tile, wtile) visit --------
            def emit_warm(xb, tt, wi):
                csz = WTILE_COLS[wi]
                pi = WTILE_POOL[wi]
                a = wt_off[wi] - pool_lo[pi]
                ps = ps_pools[pi].tile([P, pool_w[pi]], mybir.dt.float32,
                                       tag=f"mm{pi}", name=f"mm{pi}")
                for ko in range(KO):
                    nc.tensor.matmul(
                        ps[:, a : a + csz],
                        lhsT=xb[:, ko, :], rhs=wbt[wi][:, ko, :],
                        start=(ko == 0), stop=(ko == KO - 1),
                    )
                st = stgp.tile([P, 352], mybir.dt.float32, tag="st")
                nc.scalar.copy(st[:, :csz], ps[:, a : a + csz])
                t0 = tt * P
                c0 = wt_off[wi]
                nc.sync.dma_start(
                    out[t0 : t0 + P, c0 : c0 + csz], st[:, :csz]
                )

            # ------------- steady visit: full t-tile ----------------------
            def emit_tile(tt):
                xb = load_x(tt, xin)
                t0 = tt * P
                ot = outp.tile([P, O_SHARD], mybir.dt.float32, tag="ot")
                for pi in range(3):
                    wis = [j for j in range(len(WTILE_COLS))
                           if WTILE_POOL[j] == pi]
                    ps = ps_pools[pi].tile(
                        [P, pool_w[pi]], mybir.dt.float32,
                        tag=f"mm{pi}", name=f"mm{pi}",
                    )
                    for ko in range(KO):
                        for wi in wis:
                            a = wt_off[wi] - pool_lo[pi]
                            nc.tensor.matmul(
                                ps[:, a : a + WTILE_COLS[wi]],
                                lhsT=xb[:, ko, :],
                                rhs=wbt[wi][:, ko, :],
                                start=(ko == 0 and wi == wis[0]),
                                stop=(ko == KO - 1),
                            )
                    nc.scalar.copy(
                        ot[:, pool_lo[pi] : pool_lo[pi] + pool_w[pi]], ps
                    )
                nc.sync.dma_start(out[t0 : t0 + P], ot)

            # ------------- emission schedule -------------
            xbw = [load_x(t, xwarm) for t in range(WARM_T)]
            for wi in range(len(WTILE_COLS)):
                for oi in wt_otiles[wi]:
                    emit_deq(oi)
                for t in range(WARM_T):
                    emit_warm(xbw[t], t, wi)
            for tt in range(WARM_T, n_ttiles):
                emit_tile(tt)

    nc.compile()
    return nc


def _run(nc, in_maps, trace=False):
    global LAST_RESULT
    res = run_bass_kernel_spmd(
        nc, in_maps, core_ids=list(range(len(in_maps))), trace=trace
    )
    LAST_RESULT = res
    return res


def pack_x(x2d):
    """[T, K] -> packed fp16: H[tt*P+p, ko*G+t] = x2d[tt*P+t, ko*G+p]."""
    T, K = x2d.shape
    x4 = x2d.reshape(T // P, P, K // GROUP, GROUP)  # [tt, t, ko, p]
    return np.ascontiguousarray(
        x4.transpose(0, 3, 2, 1).reshape(T, K).astype(np.float16)
    )


def kernel(x, weight):
    T = FULL_B * FULL_S
    K = FULL_K
    OS = FULL_O // N_CORES  # 1376
    x2d = pack_x(np.asarray(x, dtype=np.float32).reshape(T, K))
    w = np.asarray(weight, dtype=np.float32)

    nc = build_program(K, T, OS)
    in_maps = [
        {"xt": x2d, "w": np.ascontiguousarray(w[c * OS : (c + 1) * OS])}
        for c in range(N_CORES)
    ]
    trace = bool(os.environ.get("BASS_TRACE"))
    res = _run(nc, in_maps, trace=trace)
    full = np.concatenate(
        [res.results[c]["out"] for c in range(N_CORES)], axis=1
    )
    return np.ascontiguousarray(full.reshape(FULL_B, FULL_S, FULL_O))


# revision 7
# speedup vs baseline: 1.0004x; 1.0004x over previous
"""BitLinear (ternary-quantized linear) Trainium2 kernel.

Computes: out = x @ dequant(weight).T where dequant is per-group(128)
AbsMean ternary quantization (w_q in {-1,0,+1} times per-group scale).

Strategy (8 NeuronCores, column-parallel / tensor-parallel):
  - weight [O=11008, K=4096] sharded by rows across 8 cores (1376 each).
  - x [T=8192, K] replicated, host-packed to the transposed tile layout
    AND host-cast to fp16, so each t-tile loads as one contiguous 1MB
    DMA straight into the matmul operand layout (no on-chip cast).
  - On-chip dequant per o-tile (128 rows), engine-balanced:
      DVE:    per-group abs-sum reduce, thresholds, c+ = (w > s/2),
              d = c+ - c-  (ternary in fp16)
      GpSimd: c- = (w < -s/2), w_eff = d * fp16(s) broadcast mult
      PE:     [o,k]->[k,o] transposes (identity matmul)
      ACT:    transpose evacuations into the resident ko-major tiles
  - Resident weight: 5 ko-major fp16 tiles of 256/256/256/256/352
    columns (rhs streams of 256+ cols hide LDWEIGHTS).
  - Warmup: as each weight tile finishes dequant, it is swept over the
    first WARM_T t-tiles (psum-accumulated per (t, wtile), partial-col
    DMA out).  No catch-up visits: by the time steady visits start all
    weight tiles are resident, so t-tiles 4..63 are single full visits.
  - Steady visit: per psum pool, pool-contiguous matmul runs
    (ko-major within the pool) to minimize pool switching; PSUM: two
    256-wide tiles accumulate into one 2KB bank; the second uses
    start=False at ko==0 and lands on the pending-zeroed bank.
  - Per-core output [T, 1376]; host concatenates along O.
"""

import os

import numpy as np

import concourse.bass as bass
import concourse.mybir as mybir
import concourse.tile as tile
from concourse import bacc
from concourse.bass_utils import run_bass_kernel_spmd
from concourse.masks import make_identity

P = 128
GROUP = 128
EPS = 1e-8
TB = 8

FULL_B, FULL_S, FULL_K, FULL_O = 4, 2048, 4096, 11008
N_CORES = 8

LAST_RESULT = None  # BassKernelResults of the most recent run (for test.py)

# Weight-tile column widths (>=256 so rhs streams hide LDWEIGHTS).
WTILE_COLS = [256, 256, 256, 256, 352]
# psum pool index for each weight tile (two 256s share one bank).
WTILE_POOL = [0, 0, 1, 1, 2]
# number of t-tiles each weight tile is swept over during warmup
WARM_T = 3


def build_program(K, T, O_SHARD, mm_dt=mybir.dt.float16):
    assert K % GROUP == 0 and T % P == 0
    KO = K // GROUP
    KH = KO // 2
    n_ttiles = T // P
    o_tiles = [(o0, min(P, O_SHARD - o0)) for o0 in range(0, O_SHARD, P)]
    n_ot = len(o_tiles)
    assert sum(WTILE_COLS) == O_SHARD
    wt_off = [sum(WTILE_COLS[:i]) for i in range(len(WTILE_COLS))]
    # o-tiles belonging to each weight tile
    wt_otiles = [
        [i for i, (o0, _) in enumerate(o_tiles)
         if c0 <= o0 < c0 + csz]
        for c0, csz in zip(wt_off, WTILE_COLS)
    ]

    nc = bacc.Bacc("TRN2", target_bir_lowering=False, debug=False)
    xt = nc.dram_tensor("xt", [T, K], mm_dt, kind="ExternalInput").ap()
    w = nc.dram_tensor(
        "w", [O_SHARD, K], mybir.dt.float32, kind="ExternalInput"
    ).ap()
    out = nc.dram_tensor(
        "out", [T, O_SHARD], mybir.dt.float32, kind="ExternalOutput"
    ).ap()

    with tile.TileContext(nc) as tc:
        with (
            tc.tile_pool(name="wres", bufs=1) as wres,
            tc.tile_pool(name="const", bufs=1) as constp,
            tc.tile_pool(name="deq32", bufs=3) as deq32,
            tc.tile_pool(name="deq16", bufs=5) as deq16,
            tc.tile_pool(name="tiny", bufs=3) as tiny,
            tc.tile_pool(name="xwarm", bufs=WARM_T) as xwarm,
            tc.tile_pool(name="xin", bufs=2) as xin,
            tc.tile_pool(name="outp", bufs=2) as outp,
            tc.tile_pool(name="stg", bufs=2) as stgp,
            tc.tile_pool(name="ps_tp", bufs=2, space="PSUM") as ps_tp,
            tc.tile_pool(name="ps_a", bufs=2, space="PSUM") as ps_a,
            tc.tile_pool(name="ps_b", bufs=2, space="PSUM") as ps_b,
            tc.tile_pool(name="ps_c", bufs=2, space="PSUM") as ps_c,
        ):
            ps_pools = [ps_a, ps_b, ps_c]
            pool_w = [512, 512, 352]
            pool_lo = [0, 512, 1024]
            # Resident dequantized transposed weight, ko-major per wtile:
            # wbt[wi][p, ko, col] with contiguous columns per ko (fast rhs).
            wbt = [
                wres.tile([P, KO, csz], mm_dt, tag=f"wbt{wi}", name=f"wbt{wi}")
                for wi, csz in enumerate(WTILE_COLS)
            ]
            ident = constp.tile([P, P], mm_dt)
            make_identity(nc, ident)

            # ------------- dequant of one o-tile (in ko-halves) -----------
            def emit_deq(i):
                o0, osz = o_tiles[i]
                wi = next(
                    j for j, c0 in enumerate(wt_off)
                    if c0 <= o0 < c0 + WTILE_COLS[j]
                )
                lo = o0 - wt_off[wi]
                wsrc = w[o0 : o0 + osz].rearrange("o (ko k) -> o ko k", k=GROUP)
                for h in range(2):
                    ka = h * KH
                    wt = deq32.tile([P, KH, GROUP], mybir.dt.float32,
                                    tag="wt", name="wt")
                    cp = deq16.tile([P, KH, GROUP], mm_dt, tag="cp", name="cp")
                    cm = deq16.tile([P, KH, GROUP], mm_dt, tag="cm", name="cm")
                    sums = tiny.tile([P, KH], mybir.dt.float32, tag="sums")
                    tpos = tiny.tile([P, KH], mybir.dt.float32, tag="tpos")
                    tneg = tiny.tile([P, KH], mybir.dt.float32, tag="tneg")
                    s16 = tiny.tile([P, KH], mm_dt, tag="s16")
                    nc.sync.dma_start(wt[:osz], wsrc[:, ka : ka + KH])
                    nc.vector.tensor_reduce(
                        sums[:osz], wt[:osz],
                        axis=mybir.AxisListType.X, op=mybir.AluOpType.add,
                        apply_absolute_value=True,
                    )
                    nc.vector.tensor_scalar(
                        tpos[:osz], sums[:osz], 0.5 / GROUP, 0.5 * EPS,
                        mybir.AluOpType.mult, mybir.AluOpType.max,
                    )
                    nc.vector.tensor_scalar(
                        tneg[:osz], sums[:osz], -0.5 / GROUP, -0.5 * EPS,
                        mybir.AluOpType.mult, mybir.AluOpType.min,
                    )
                    nc.vector.tensor_scalar(
                        s16[:osz], sums[:osz], 1.0 / GROUP, EPS,
                        mybir.AluOpType.mult, mybir.AluOpType.max,
                    )
                    nc.vector.tensor_tensor(
                        cp[:osz], wt[:osz],
                        tpos[:osz, :, None].to_broadcast((osz, KH, GROUP)),
                        mybir.AluOpType.is_gt,
                    )
                    nc.vector.tensor_tensor(
                        cm[:osz], wt[:osz],
                        tneg[:osz, :, None].to_broadcast((osz, KH, GROUP)),
                        mybir.AluOpType.is_lt,
                    )
                    nc.vector.tensor_tensor(
                        cp[:osz], cp[:osz], cm[:osz],
                        mybir.AluOpType.subtract,
                    )
                    nc.gpsimd.tensor_tensor(
                        cp[:osz], cp[:osz],
                        s16[:osz, :, None].to_broadcast((osz, KH, GROUP)),
                        mybir.AluOpType.mult,
                    )
                    for g in range(KH // TB):
                        kb = ka + g * TB
                        ps = ps_tp.tile([P, TB, P], mm_dt, tag="tp")
                        for j in range(TB):
                            nc.tensor.transpose(
                                ps[:, j, :osz], cp[:osz, g * TB + j, :],
                                ident[:osz, :osz],
                            )
                        nc.scalar.copy(
                            wbt[wi][:, kb : kb + TB, lo : lo + osz],
                            ps[:, :, :osz],
                        )

            # ------------- x tile load ------------------------------------
            xt_r = xt.rearrange("(tt p) (ko t) -> tt p ko t", p=P, t=P)

            def load_x(tt, pool):
                xb = pool.tile([P, KO, P], mm_dt, tag="xb")
                nc.sync.dma_start(xb, xt_r[tt])
                return xb

            # ------------- warmup sweep: one (t-tile, wtile) visit --------
            def emit_warm(xb, tt, wi):
                csz = WTILE_COLS[wi]
                pi = WTILE_POOL[wi]
                a = wt_off[wi] - pool_lo[pi]
                ps = ps_pools[pi].tile([P, pool_w[pi]], mybir.dt.float32,
                                       tag=f"mm{pi}", name=f"mm{pi}")
                for ko in range(KO):
                    nc.tensor.matmul(
                        ps[:, a : a + csz],
                        lhsT=xb[:, ko, :], rhs=wbt[wi][:, ko, :],
                        start=(ko == 0), stop=(ko == KO - 1),
                    )
                st = stgp.tile([P, 352], mybir.dt.float32, tag="st")
                nc.scalar.copy(st[:, :csz], ps[:, a : a + csz])
                t0 = tt * P
                c0 = wt_off[wi]
                nc.sync.dma_start(
                    out[t0 : t0 + P, c0 : c0 + csz], st[:, :csz]
                )

            # ------------- steady visit: full t-tile ----------------------
            def emit_tile(tt):
                xb = load_x(tt, xin)
                t0 = tt * P
                ot = outp.tile([P, O_SHARD], mybir.dt.float32, tag="ot")
                for pi in range(3):
                    wis = [j for j in range(len(WTILE_COLS))
                           if WTILE_POOL[j] == pi]
                    ps = ps_pools[pi].tile(
                        [P, pool_w[pi]], mybir.dt.float32,
                        tag=f"mm{pi}", name=f"mm{pi}",
                    )
                    for ko in range(KO):
                        for wi in wis:
                            a = wt_off[wi] - pool_lo[pi]
                            nc.tensor.matmul(
                                ps[:, a : a + WTILE_COLS[wi]],
                                lhsT=xb[:, ko, :],
                                rhs=wbt[wi][:, ko, :],
                                start=(ko == 0 and wi == wis[0]),
                                stop=(ko == KO - 1),
                            )
                    nc.scalar.copy(
                        ot[:, pool_lo[pi] : pool_lo[pi] + pool_w[pi]], ps
                    )
                nc.sync.dma_start(out[t0 : t0 + P], ot)

            # ------------- emission schedule -------------
            for oi in wt_otiles[0]:
                emit_deq(oi)
            xbw = [load_x(t, xwarm) for t in range(WARM_T)]
            for wi in range(len(WTILE_COLS)):
                if wi > 0:
                    for oi in wt_otiles[wi]:
                        emit_deq(oi)
                for t in range(WARM_T):
                    emit_warm(xbw[t], t, wi)
            for tt in range(WARM_T, n_ttiles):
                emit_tile(tt)

    nc.compile()
    return nc


def _run(nc, in_maps, trace=False):
    global LAST_RESULT
    res = run_bass_kernel_spmd(
        nc, in_maps, core_ids=list(range(len(in_maps))), trace=trace
    )
    LAST_RESULT = res
    return res


def pack_x(x2d):
    """[T, K] -> packed fp16: H[tt*P+p, ko*G+t] = x2d[tt*P+t, ko*G+p]."""
    T, K = x2d.shape
    x4 = x2d.reshape(T // P, P, K // GROUP, GROUP)  # [tt, t, ko, p]
    return np.ascontiguousarray(
        x4.transpose(0, 3, 2, 1).reshape(T, K).astype(np.float16)
    )


def kernel(x, weight):
    T = FULL_B * FULL_S
    K = FULL_K
    OS = FULL_O // N_CORES  # 1376
    x2d = pack_x(np.asarray(x, dtype=np.float32).reshape(T, K))
    w = np.asarray(weight, dtype=np.float32)

    nc = build_program(K, T, OS)
    in_maps = [
        {"xt": x2d, "w": np.ascontiguousarray(w[c * OS : (c + 1) * OS])}
        for c in range(N_CORES)
    ]
    trace = bool(os.environ.get("BASS_TRACE"))
    res = _run(nc, in_maps, trace=trace)
    full = np.concatenate(
        [res.results[c]["out"] for c in range(N_CORES)], axis=1
    )
    return np.ascontiguousarray(full.reshape(FULL_B, FULL_S, FULL_O))


# revision 8
# speedup vs baseline: 1.0507x; 1.0503x over previous
"""BitLinear (ternary-quantized linear) Trainium2 kernel.

Computes: out = x @ dequant(weight).T where dequant is per-group(128)
AbsMean ternary quantization (w_q in {-1,0,+1} times per-group scale).

Strategy (8 NeuronCores, column-parallel / tensor-parallel):
  - weight [O=11008, K=4096] sharded by rows across 8 cores (1376 each).
  - x [T=8192, K] replicated, host-packed to the transposed tile layout
    AND host-cast to fp16, so each t-tile loads as one contiguous 1MB
    DMA straight into the matmul operand layout (no on-chip cast).
  - On-chip dequant per o-tile (128 rows), engine-balanced around the
    measured rates (DVE fp32 ~115G elem/s, ACT ~118G, GpSimd ~73G):
      ACT: sgn=Sign(w), |w| (in-place);  DVE: abs-sum reduce,
      c=(|w|>s/2), cs=c*sgn (alternating halves on GpSimd);
      GpSimd: w_eff=cs*fp16(s);  PE: [o,k]->[k,o] transposes;
      evac of transposes alternates ACT/DVE.
  - Resident weight: 5 ko-major fp16 tiles of 256/256/256/256/352 cols.
  - Adaptive warmup: a simulated production clock (dequant finishes an
    o-tile every ~PROD ns) paces early t-tile visits, each covering the
    column prefix that is ready when it would issue; the not-yet-ready
    suffix of those tiles is covered by catch-up visits interleaved
    into the steady phase.
  - Visits run pool-contiguous matmul runs (ko-major per weight tile),
    evacuate per psum pool into small staging tiles and DMA the pool's
    column span directly (no full-row staging buffer).
  - Per-core output [T, 1376]; host concatenates along O.
"""

import os
from bisect import bisect_right

import numpy as np

import concourse.bass as bass
import concourse.mybir as mybir
import concourse.tile as tile
from concourse import bacc
from concourse.bass_utils import run_bass_kernel_spmd
from concourse.masks import make_identity

P = 128
GROUP = 128
EPS = 1e-8
TB = 8

FULL_B, FULL_S, FULL_K, FULL_O = 4, 2048, 4096, 11008
N_CORES = 8

LAST_RESULT = None  # BassKernelResults of the most recent run (for test.py)

# Weight-tile column widths (>=256 so rhs streams hide LDWEIGHTS).
WTILE_COLS = [256, 256, 256, 256, 352]
# psum pool index for each weight tile (two 256s share one bank).
WTILE_POOL = [0, 0, 1, 1, 2]

# warmup pacing model (ns): o-tile dequant production rate, first-ready
# latency, per-visit fixed overhead, per-column matmul cost (32 ko).
PROD = 13000
LAT0 = 16000
VISIT_OH = 400
COLT = 13.34
CATCHUP_EVERY = 3


def build_program(K, T, O_SHARD, mm_dt=mybir.dt.float16):
    assert K % GROUP == 0 and T % P == 0
    KO = K // GROUP
    KH = KO // 2
    n_ttiles = T // P
    o_tiles = [(o0, min(P, O_SHARD - o0)) for o0 in range(0, O_SHARD, P)]
    n_ot = len(o_tiles)
    assert sum(WTILE_COLS) == O_SHARD
    wt_off = [sum(WTILE_COLS[:i]) for i in range(len(WTILE_COLS))]

    nc = bacc.Bacc("TRN2", target_bir_lowering=False, debug=False)
    xt = nc.dram_tensor("xt", [T, K], mm_dt, kind="ExternalInput").ap()
    w = nc.dram_tensor(
        "w", [O_SHARD, K], mybir.dt.float32, kind="ExternalInput"
    ).ap()
    out = nc.dram_tensor(
        "out", [T, O_SHARD], mybir.dt.float32, kind="ExternalOutput"
    ).ap()

    with tile.TileContext(nc) as tc:
        with (
            tc.tile_pool(name="wres", bufs=1) as wres,
            tc.tile_pool(name="const", bufs=1) as constp,
            tc.tile_pool(name="deq32", bufs=3) as deq32,
            tc.tile_pool(name="deq16", bufs=4) as deq16,
            tc.tile_pool(name="tiny", bufs=8) as tiny,
            tc.tile_pool(name="xin", bufs=4) as xin,
            tc.tile_pool(name="stg", bufs=4) as stgp,
            tc.tile_pool(name="ps_tp", bufs=2, space="PSUM") as ps_tp,
            tc.tile_pool(name="ps_a", bufs=2, space="PSUM") as ps_a,
            tc.tile_pool(name="ps_b", bufs=2, space="PSUM") as ps_b,
            tc.tile_pool(name="ps_c", bufs=2, space="PSUM") as ps_c,
        ):
            ps_pools = [ps_a, ps_b, ps_c]
            pool_w = [512, 512, 352]
            pool_lo = [0, 512, 1024]
            # Resident dequantized transposed weight, ko-major per wtile:
            # wbt[wi][p, ko, col] with contiguous columns per ko (fast rhs).
            wbt = [
                wres.tile([P, KO, csz], mm_dt, tag=f"wbt{wi}", name=f"wbt{wi}")
                for wi, csz in enumerate(WTILE_COLS)
            ]
            ident = constp.tile([P, P], mm_dt)
            make_identity(nc, ident)

            # ------------- dequant of one o-tile (in ko-halves) -----------
            def emit_deq(i):
                o0, osz = o_tiles[i]
                wi = next(
                    j for j, c0 in enumerate(wt_off)
                    if c0 <= o0 < c0 + WTILE_COLS[j]
                )
                lo = o0 - wt_off[wi]
                wsrc = w[o0 : o0 + osz].rearrange("o (ko k) -> o ko k", k=GROUP)
                for h in range(2):
                    ka = h * KH
                    wt = deq32.tile([P, KH, GROUP], mybir.dt.float32,
                                    tag="wt", name="wt")
                    sgn = deq16.tile([P, KH, GROUP], mm_dt, tag="sgn",
                                     name="sgn")
                    c = deq16.tile([P, KH, GROUP], mm_dt, tag="c", name="c")
                    sums = tiny.tile([P, KH], mybir.dt.float32, tag="sums")
                    tpos = tiny.tile([P, KH], mybir.dt.float32, tag="tpos")
                    s16 = tiny.tile([P, KH], mm_dt, tag="s16")
                    nc.sync.dma_start(wt[:osz], wsrc[:, ka : ka + KH])
                    nc.vector.tensor_reduce(
                        sums[:osz], wt[:osz],
                        axis=mybir.AxisListType.X, op=mybir.AluOpType.add,
                        apply_absolute_value=True,
                    )
                    nc.vector.tensor_scalar(
                        tpos[:osz], sums[:osz], 0.5 / GROUP, 0.5 * EPS,
                        mybir.AluOpType.mult, mybir.AluOpType.max,
                    )
                    nc.vector.tensor_scalar(
                        s16[:osz], sums[:osz], 1.0 / GROUP, EPS,
                        mybir.AluOpType.mult, mybir.AluOpType.max,
                    )
                    nc.scalar.activation(
                        sgn[:osz], wt[:osz],
                        mybir.ActivationFunctionType.Sign,
                    )
                    nc.scalar.activation(
                        wt[:osz], wt[:osz],
                        mybir.ActivationFunctionType.Abs,
                    )
                    nc.vector.tensor_tensor(
                        c[:osz], wt[:osz],
                        tpos[:osz, :, None].to_broadcast((osz, KH, GROUP)),
                        mybir.AluOpType.is_gt,
                    )
                    # cs = c*sgn: alternate DVE / GpSimd per half
                    eng = nc.vector if h == 0 else nc.gpsimd
                    eng.tensor_tensor(
                        c[:osz], c[:osz], sgn[:osz], mybir.AluOpType.mult,
                    )
                    nc.gpsimd.tensor_tensor(
                        c[:osz], c[:osz],
                        s16[:osz, :, None].to_broadcast((osz, KH, GROUP)),
                        mybir.AluOpType.mult,
                    )
                    for g in range(KH // TB):
                        kb = ka + g * TB
                        ps = ps_tp.tile([P, TB, P], mm_dt, tag="tp")
                        for j in range(TB):
                            nc.tensor.transpose(
                                ps[:, j, :osz], c[:osz, g * TB + j, :],
                                ident[:osz, :osz],
                            )
                        dst = wbt[wi][:, kb : kb + TB, lo : lo + osz]
                        if g == 0:
                            nc.scalar.copy(dst, ps[:, :, :osz])
                        else:
                            nc.vector.tensor_copy(dst, ps[:, :, :osz])

            # ------------- x tile load ------------------------------------
            xt_r = xt.rearrange("(tt p) (ko t) -> tt p ko t", p=P, t=P)

            def load_x(tt):
                xb = xin.tile([P, KO, P], mm_dt, tag="xb")
                nc.sync.dma_start(xb, xt_r[tt])
                return xb

            # ------------- one visit over a column range ------------------
            def emit_visit(tt, col_lo, col_hi):
                xb = load_x(tt)
                t0 = tt * P
                for pi in range(3):
                    a = max(col_lo, pool_lo[pi])
                    b = min(col_hi, pool_lo[pi] + pool_w[pi])
                    if a >= b:
                        continue
                    spans = []
                    for wi in range(len(WTILE_COLS)):
                        if WTILE_POOL[wi] != pi:
                            continue
                        wa = max(a, wt_off[wi]) - wt_off[wi]
                        wb_ = min(b, wt_off[wi] + WTILE_COLS[wi]) - wt_off[wi]
                        if wa < wb_:
                            spans.append((wi, wa, wb_))
                    ps = ps_pools[pi].tile(
                        [P, pool_w[pi]], mybir.dt.float32,
                        tag=f"mm{pi}", name=f"mm{pi}",
                    )
                    for si, (wi, wa, wb_) in enumerate(spans):
                        pa = wt_off[wi] - pool_lo[pi]
                        for ko in range(KO):
                            nc.tensor.matmul(
                                ps[:, pa + wa : pa + wb_],
                                lhsT=xb[:, ko, :],
                                rhs=wbt[wi][:, ko, wa:wb_],
                                start=(ko == 0 and si == 0),
                                stop=(ko == KO - 1),
                            )
                    st = stgp.tile([P, 512], mybir.dt.float32, tag="st")
                    nc.scalar.copy(
                        st[:, : b - a], ps[:, a - pool_lo[pi] : b - pool_lo[pi]]
                    )
                    nc.sync.dma_start(
                        out[t0 : t0 + P, a:b], st[:, : b - a]
                    )

            # ------------- emission schedule -------------
            # simulate dequant production to pace the warmup visits
            ready_t = [LAT0 + PROD * i for i in range(n_ot)]
            events = [(ready_t[i] - PROD, 0, ("deq", i)) for i in range(n_ot)]
            clock = float(ready_t[0])
            tt = 0
            warm = []
            while True:
                cols = min(128 * bisect_right(ready_t, clock), O_SHARD)
                if cols >= O_SHARD:
                    break
                if cols == 0:
                    clock = float(ready_t[0])
                    continue
                events.append((clock, 1, ("visit", (tt, cols))))
                warm.append((tt, cols))
                clock += cols * COLT + VISIT_OH
                tt += 1
            for _, _, (kind, arg) in sorted(events, key=lambda e: (e[0], e[1])):
                if kind == "deq":
                    emit_deq(arg)
                else:
                    emit_visit(arg[0], 0, arg[1])

            catchups = list(warm)
            ci = 0
            for k, t in enumerate(range(tt, n_ttiles)):
                emit_visit(t, 0, O_SHARD)
                if (k + 1) % CATCHUP_EVERY == 0 and ci < len(catchups):
                    jt, jc = catchups[ci]
                    emit_visit(jt, jc, O_SHARD)
                    ci += 1
            while ci < len(catchups):
                jt, jc = catchups[ci]
                emit_visit(jt, jc, O_SHARD)
                ci += 1

    nc.compile()
    return nc


def _run(nc, in_maps, trace=False):
    global LAST_RESULT
    res = run_bass_kernel_spmd(
        nc, in_maps, core_ids=list(range(len(in_maps))), trace=trace
    )
    LAST_RESULT = res
    return res


def pack_x(x2d):
    """[T, K] -> packed fp16: H[tt*P+p, ko*G+t] = x2d[tt*P+t, ko*G+p]."""
    T, K = x2d.shape
    x4 = x2d.reshape(T // P, P, K // GROUP, GROUP)  # [tt, t, ko, p]
    return np.ascontiguousarray(
        x4.transpose(0, 3, 2, 1).reshape(T, K).astype(np.float16)
    )


def kernel(x, weight):
    T = FULL_B * FULL_S
    K = FULL_K
    OS = FULL_O // N_CORES  # 1376
    x2d = pack_x(np.asarray(x, dtype=np.float32).reshape(T, K))
    w = np.asarray(weight, dtype=np.float32)

    nc = build_program(K, T, OS)
    in_maps = [
        {"xt": x2d, "w": np.ascontiguousarray(w[c * OS : (c + 1) * OS])}
        for c in range(N_CORES)
    ]
    trace = bool(os.environ.get("BASS_TRACE"))
    res = _run(nc, in_maps, trace=trace)
    full = np.concatenate(
        [res.results[c]["out"] for c in range(N_CORES)], axis=1
    )
    return np.ascontiguousarray(full.reshape(FULL_B, FULL_S, FULL_O))
